# revision 5
# baseline (speedup 1.0000x reference)
"""Trainium2 Bass kernel for nn_MultiHeadAttention (B=2, S=2048, D=2048, H=16, Dh=128).

Sharding: tensor-parallel over heads — 2 heads per core on 8 cores. Each core
computes q/k/v projections for its 2 heads, RoPE, causal attention, and a
partial output projection against its 256-column slice of Wo; the host sums
the 8 partial outputs.

v2 structure (vs the v1 two-phase design):
 - Phase-interleaved emission: batch-1 projections (PE-dense) are woven
   between batch-0 attention rounds (scalar-dense) so the tensor engine
   never starves while the activation engine works through the softmax exps.
   Output projection (stage_c) chunks are likewise spread as PE filler.
 - RoPE runs in bf16 on SBUF (DVE 2x mode) after one scalar copy from PSUM.
 - All 4 per-tile transposes (q0,q1,k0,k1) land in ONE psum bank and drain
   with ONE strided DVE copy into a combined qkT tile.
 - Softmax epilogue: ones-matrix matmul broadcasts the denominator row to
   all 128 partitions in one instruction; reciprocal + one DVE multiply
   finish the normalization (no per-stream broadcast matmul / copies).
"""

import math
import sys

import numpy as np

try:
    import concourse.bass as bass
except ImportError:  # pragma: no cover
    sys.path.insert(0, "/opt/trn_rl_repo")
    import concourse.bass as bass

import ml_dtypes
import concourse.mybir as mybir
import concourse.tile as tile
from concourse import bacc
from concourse.bass_utils import run_bass_kernel_spmd
from concourse.masks import make_identity

F32 = mybir.dt.float32
BF16 = mybir.dt.bfloat16
F16 = mybir.dt.float16

B, S, D = 2, 2048, 2048
H, DH = 16, 128
N_CORES = 8
HPC = H // N_CORES  # 2 heads per core
T = B * S  # 4096
TT = S // 128  # 16 token tiles per batch
SCALE = 1.0 / math.sqrt(DH)

# PE-time pacing estimates (ns) for interleaving only — not correctness.
A_UNIT_PE = 5330.0
C_UNIT_PE = 1704.0


def build_nc(reps=1, av_depth=1, xbufs=3, pbufs=6, stages="abc"):
    nc = bacc.Bacc("TRN2", target_bir_lowering=False, debug=False,
                   num_devices=N_CORES)

    XT = nc.dram_tensor("XT", [T // 256, 128, 16, 256], BF16, kind="ExternalInput")
    WALL = nc.dram_tensor("WALL", [D, 768], BF16, kind="ExternalInput")
    W2 = nc.dram_tensor("W2", [2 * DH, D], BF16, kind="ExternalInput")
    C4 = nc.dram_tensor("C4", [S, 256], BF16, kind="ExternalInput")
    S4 = nc.dram_tensor("S4", [S, 256], BF16, kind="ExternalInput")
    LM = nc.dram_tensor("LM", [128, 128], BF16, kind="ExternalInput")
    ONES = nc.dram_tensor("ONES", [128, 128], BF16, kind="ExternalInput")
    Y = nc.dram_tensor("Y", [T, D], F16, kind="ExternalOutput")

    with nc.allow_low_precision(reason="bf16 matmul/rope internals"), \
         tile.TileContext(nc) as tc:
        with tc.tile_pool(name="res", bufs=1) as res, \
             tc.tile_pool(name="work", bufs=2) as work, \
             tc.tile_pool(name="psA", bufs=2, space="PSUM") as psA, \
             tc.tile_pool(name="psB", bufs=2, space="PSUM") as psB, \
             tc.tile_pool(name="psO", bufs=2, space="PSUM") as psO, \
             tc.tile_pool(name="psC", bufs=2, space="PSUM") as psC:

            # ---- resident tensors (spread across DMA queues) ----
            wall_sb = res.tile([128, 16, 768], BF16)
            WALL_r = WALL.rearrange("(dk p) f -> p dk f", p=128)
            for dk in range(16):
                nc.scalar.dma_start(wall_sb[:, dk, :], WALL_r[:, dk, :])
            c4_sb = res.tile([128, TT, 256], BF16)
            s4_sb = res.tile([128, TT, 256], BF16)
            C4_r = C4.rearrange("(tt p) j -> p tt j", p=128)
            S4_r = S4.rearrange("(tt p) j -> p tt j", p=128)
            lm_sb = res.tile([128, 128], BF16)
            ones_sb = res.tile([128, 128], BF16)

            def load_tables_head():
                # small leading pieces right after the first x columns on the
                # sync queue: tt0-3 rope tables land before they are needed
                nc.sync.dma_start(c4_sb[:, 0:4, :], C4_r[:, 0:4, :])
                nc.sync.dma_start(s4_sb[:, 0:4, :], S4_r[:, 0:4, :])
                nc.sync.dma_start(lm_sb[:], LM[:])
                nc.sync.dma_start(ones_sb[:], ONES[:])

            def load_tables_bulk():
                nc.scalar.dma_start(c4_sb[:, 4:TT, :], C4_r[:, 4:TT, :])
                nc.scalar.dma_start(s4_sb[:, 4:TT, :], S4_r[:, 4:TT, :])
            ident = res.tile([128, 128], BF16)
            make_identity(nc, ident[:])
            load_tables_bulk()
            # warm the Exp activation table off the critical path (after the
            # wall DMA issues so it doesn't delay them)
            warm = res.tile([1, 2], BF16)
            nc.scalar.activation(warm[:], lm_sb[0:1, 0:2],
                                 mybir.ActivationFunctionType.Exp)
            w2_sb = res.tile([128, 2, D], BF16)

            def load_w2():
                nc.scalar.dma_start(
                    w2_sb[:], W2.rearrange("(h p) e -> p h e", p=128))

            def batch_tiles(b):
                # qkT groups: 0,1 = q heads 0,1; 2,3 = k heads 0,1
                qkT = work.tile([128, 4, S], BF16, tag=f"qkT{b}", bufs=1,
                                name=f"qkT{b}")
                v_sb = work.tile([128, TT, 256], BF16, tag=f"v{b}", bufs=1,
                                 name=f"v{b}")
                outT = work.tile([128, HPC, S], BF16, tag=f"outT{b}", bufs=1,
                                 name=f"outT{b}")
                return qkT, v_sb, outT

            def stage_a_units(b, qkT, v_sb):
                """Generator: one yield per token tile (32 proj matmuls +
                rope + transposes)."""
                xcols = {}
                for t2 in range(TT // 2):
                    xc = work.tile([128, 16, 256], BF16, tag="xcol", bufs=xbufs,
                                   name=f"xcol{t2}")
                    if b == 0 and t2 == 0:
                        # split along dk (contiguous runs) so the first
                        # contraction chunks land sooner
                        nc.sync.dma_start(xc[:, 0:2, :], XT[0][:, 0:2, :])
                        nc.sync.dma_start(xc[:, 2:16, :], XT[0][:, 2:16, :])
                        load_tables_head()
                    else:
                        nc.sync.dma_start(xc[:], XT[b * (TT // 2) + t2])
                    xcols[t2] = xc
                def qkv_tiles(tt, fused_with=None):
                    """Issue the 32 projection matmuls for tile tt (and, when
                    fused_with is set, interleave per-dk with a second tile so
                    the two consume each arriving wall chunk together)."""
                    tts = [tt] + ([fused_with] if fused_with is not None else [])
                    ps = {}
                    for i, t in enumerate(tts):
                        pq = (psA if (b == 0 and t % 2 == 0) else
                              (psC if b == 0 else psA))
                        pv = (psB if (b == 0 and t % 2 == 0) else
                              (psO if b == 0 else psB))
                        ps[t] = (pq.tile([128, 512], F32,
                                         tag="qk" if pq is psA else "s",
                                         name=f"ps_qk{t % 2}"),
                                 pv.tile([128, 256], F32,
                                         tag="tv" if pv is psB else "o",
                                         name=f"ps_v{t % 2}"))
                    for dk in range(16):
                        for t in tts:
                            xcol = xcols[t // 2][
                                :, :, (t % 2) * 128:(t % 2) * 128 + 128]
                            nc.tensor.matmul(ps[t][0][:], xcol[:, dk, :],
                                             wall_sb[:, dk, 0:512],
                                             start=(dk == 0), stop=(dk == 15))
                            nc.tensor.matmul(ps[t][1][:], xcol[:, dk, :],
                                             wall_sb[:, dk, 512:768],
                                             start=(dk == 0), stop=(dk == 15))
                    return ps

                def finish_tile(tt, ps_pair):
                    ps_qk, ps_v = ps_pair
                    if b == 0 and tt < 2:
                        # scalar queue is still draining resident DMAs
                        nc.vector.tensor_copy(v_sb[:, tt, :], ps_v[:])
                    else:
                        nc.scalar.copy(v_sb[:, tt, :], ps_v[:])
                    qk_sb = work.tile([128, 512], BF16, tag="qks", bufs=2)
                    if b == 0 and tt < 2:
                        nc.vector.tensor_copy(qk_sb[:], ps_qk[:])
                    else:
                        nc.scalar.copy(qk_sb[:], ps_qk[:])
                    # RoPE in bf16 on SBUF: blocks g in {q0,q1,k0,k1}, each
                    # [top64 | bot64]
                    qkv = qk_sb.rearrange("p (g two j) -> p g two j", two=2, j=64)
                    topv, botv = qkv[:, :, 0, :], qkv[:, :, 1, :]
                    ct = c4_sb[:, tt, :].rearrange("p (g j) -> p g j", j=64)
                    st = s4_sb[:, tt, :].rearrange("p (g j) -> p g j", j=64)
                    m1 = work.tile([128, 4, 64], BF16, tag="m1")
                    m2 = work.tile([128, 4, 64], BF16, tag="m2")
                    rot = work.tile([128, 512], BF16, tag="rot")
                    rotv = rot.rearrange("p (g two j) -> p g two j", two=2, j=64)
                    nc.vector.tensor_mul(m1[:], topv, ct)
                    nc.vector.tensor_mul(m2[:], botv, st)
                    nc.vector.tensor_sub(rotv[:, :, 0, :], m1[:], m2[:])
                    nc.vector.tensor_mul(m1[:], botv, ct)
                    nc.vector.tensor_mul(m2[:], topv, st)
                    nc.vector.tensor_add(rotv[:, :, 1, :], m1[:], m2[:])
                    pt = psO if (b == 0 and tt % 2 == 1) else psB
                    ps_t4 = pt.tile([128, 512], BF16,
                                    tag="tv" if pt is psB else "o",
                                    name="ps_t4")
                    for g in range(4):
                        nc.tensor.transpose(ps_t4[:, g * 128:(g + 1) * 128],
                                            rot[:, g * 128:(g + 1) * 128],
                                            ident[:])
                    dst = qkT.rearrange("p g s -> p g s")[:, :, tt * 128:(tt + 1) * 128]
                    nc.vector.tensor_copy(dst, ps_t4.rearrange(
                        "p (g c) -> p g c", c=128))

                if b == 0:
                    ps = qkv_tiles(0, fused_with=1)
                    finish_tile(0, ps[0])
                    yield
                    finish_tile(1, ps[1])
                    yield
                    start = 2
                else:
                    start = 0
                for tt in range(start, TT):
                    ps = qkv_tiles(tt)
                    finish_tile(tt, ps[tt])
                    yield

            def stage_b_units(qkT, v_sb, outT, pend_c, lbc_ab=False,
                              qc_order=(0, 1, 2, 3)):
                """Generator over the 2 head-streams of one batch.
                Yields ('kt', pe_ns) after each kt row, ('round', qc) after
                each epilogue. Appends ready stage_c chunk ids to pend_c."""
                for qc in qc_order:
                    nkt = 4 * (qc + 1)
                    ps_o, acc = {}, {}
                    for h in range(HPC):
                        ps_o[h] = psO.tile([128, 512], F32, tag="o", bufs=2,
                                           name=f"ps_o{h}")
                        acc[h] = work.tile([128, 512], BF16, tag="acc", bufs=4,
                                           name=f"acc{h}")
                    pend_av = []

                    def flush_av(rounds, nkt=nkt):
                        for rnd in rounds:
                            for (fkt, foff, fh, fp) in rnd:
                                nc.tensor.matmul(
                                    ps_o[fh][:, foff:512],
                                    v_sb[:, fkt, fh * 128:(fh + 1) * 128],
                                    fp[:, foff:512],
                                    start=(fkt == 0), stop=(fkt == nkt - 1))
                    for kt in range(nkt):
                        off = max(0, (kt - 4 * qc) * 128)
                        new_av = []
                        for h in range(HPC):
                            ps_s = psC.tile([128, 512], F32, tag="s", bufs=2,
                                            name=f"ps_s{h}")
                            nc.tensor.matmul(
                                ps_s[:, off:512],
                                qkT[:, 2 + h, kt * 128:(kt + 1) * 128],
                                qkT[:, h, qc * 512 + off:(qc + 1) * 512],
                                start=True, stop=True)
                            p_sb = work.tile([128, 512], BF16, tag="p",
                                             bufs=pbufs)
                            nc.scalar.activation(p_sb[:, off:512], ps_s[:, off:512],
                                                 mybir.ActivationFunctionType.Exp,
                                                 scale=SCALE)
                            if kt >= 4 * qc:
                                nc.vector.tensor_mul(p_sb[:, off:off + 128],
                                                     p_sb[:, off:off + 128],
                                                     lm_sb[:])
                            if kt == 0:
                                nc.vector.tensor_copy(acc[h][:], p_sb[:])
                            else:
                                nc.vector.tensor_add(acc[h][:, off:512],
                                                     acc[h][:, off:512],
                                                     p_sb[:, off:512])
                            new_av.append((kt, off, h, p_sb))
                        if len(pend_av) >= av_depth:
                            flush_av([pend_av.pop(0)])
                        pend_av.append(new_av)
                        yield ("kt", 2 * (512 - off) * 0.417 * 2)
                    flush_av(pend_av)
                    pend_av = []
                    for h in range(HPC):
                        if lbc_ab:
                            pl = psA if h == 0 else psB
                            ps_l = pl.tile([128, 512], F32,
                                           tag="qk" if h == 0 else "tv",
                                           name=f"ps_l{h}")
                        else:
                            ps_l = psC.tile([128, 512], F32, tag="s", bufs=2,
                                            name=f"ps_l{h}")
                        nc.tensor.matmul(ps_l[:], ones_sb[:], acc[h][:],
                                         start=True, stop=True)
                        recipL = work.tile([128, 512], F32, tag="rc", bufs=2)
                        nc.vector.reciprocal(recipL[:], ps_l[:])
                        nc.vector.tensor_mul(outT[:, h, qc * 512:(qc + 1) * 512],
                                             ps_o[h][:], recipL[:])
                    pend_c.append(qc)
                    yield ("round", qc)

            def stage_c_unit(b, outT, tt, ps_y_tag):
                """One token tile of the output projection."""
                if "c" not in stages:
                    return
                y_sb = work.tile([128, D], F16, tag="ysb")
                if ps_y_tag == "ab":
                    # psA+psB free here: issue all 4 head-0 matmuls first so
                    # PE has runway while head-1's outT normalization drains
                    tiles = []
                    for ec in range(4):
                        if ec % 2 == 0:
                            ps_y = psA.tile([128, 512], F32, tag="qk",
                                            name="ps_y")
                        else:
                            ps_y = psB.tile([128, 512], F32, tag="tv",
                                            name="ps_y")
                        nc.tensor.matmul(ps_y[:],
                                         outT[:, 0, tt * 128:(tt + 1) * 128],
                                         w2_sb[:, 0, ec * 512:(ec + 1) * 512],
                                         start=True, stop=False)
                        tiles.append(ps_y)
                    gt0 = b * S + tt * 128
                    for ec in range(4):
                        nc.tensor.matmul(tiles[ec][:],
                                         outT[:, 1, tt * 128:(tt + 1) * 128],
                                         w2_sb[:, 1, ec * 512:(ec + 1) * 512],
                                         start=False, stop=True)
                        if ec % 2 == 0:
                            nc.scalar.copy(y_sb[:, ec * 512:(ec + 1) * 512],
                                           tiles[ec][:])
                        else:
                            nc.vector.tensor_copy(
                                y_sb[:, ec * 512:(ec + 1) * 512], tiles[ec][:])
                    nc.sync.dma_start(Y[gt0:gt0 + 128, :], y_sb[:])
                    return
                if True:
                    for ec in range(4):
                        ps_y = psC.tile([128, 512], F32, tag="s", bufs=2,
                                        name="ps_y")
                        nc.tensor.matmul(ps_y[:],
                                         outT[:, 0, tt * 128:(tt + 1) * 128],
                                         w2_sb[:, 0, ec * 512:(ec + 1) * 512],
                                         start=True, stop=False)
                        nc.tensor.matmul(ps_y[:],
                                         outT[:, 1, tt * 128:(tt + 1) * 128],
                                         w2_sb[:, 1, ec * 512:(ec + 1) * 512],
                                         start=False, stop=True)
                        if ec % 2 == 0:
                            nc.scalar.copy(y_sb[:, ec * 512:(ec + 1) * 512],
                                           ps_y[:])
                        else:
                            nc.vector.tensor_copy(
                                y_sb[:, ec * 512:(ec + 1) * 512], ps_y[:])
                gt = b * S + tt * 128
                nc.sync.dma_start(Y[gt:gt + 128, :], y_sb[:])

            def body():
                t0 = batch_tiles(0)
                t1 = batch_tiles(1)
                # phase 1: projections for batch 0 (PE-dense)
                for i, _ in enumerate(stage_a_units(0, t0[0], t0[1])):
                    if i == 8:
                        load_w2()
                # phase 2: batch-0 attention interleaved with batch-1
                # projections and ready batch-0 output-projection chunks
                if "b" not in stages:
                    for _ in stage_a_units(1, t1[0], t1[1]):
                        pass
                    return
                pend_c0 = []
                genb0 = stage_b_units(t0[0], t0[1], t0[2], pend_c0)
                gena1 = stage_a_units(1, t1[0], t1[1])
                a_left = 16
                other_pe = 0.0
                a_done_pe = 0.0
                TOTAL_B0 = 69632 * 0.417 * 2 / 2 + 4 * 2 * 213.0
                TOTAL_C_PH2 = 6 * C_UNIT_PE
                TOTAL_A = 16 * A_UNIT_PE
                c_queue = []
                c_emitted = 0
                for item in genb0:
                    if item[0] == "kt":
                        other_pe += item[1]
                    else:
                        other_pe += 2 * 213.0
                        for _ in range(4):
                            c_queue.append((0, t0[2]))
                    # pull ready stage_c chunks (cap 12 in phase 2)
                    CAP = 6
                    while c_queue and c_emitted < CAP and other_pe > (
                            (c_emitted + 1) * (TOTAL_B0 + TOTAL_C_PH2) / CAP
                            - C_UNIT_PE):
                        b_, o_ = c_queue.pop(0)
                        stage_c_unit(b_, o_, c_emitted, "s")
                        c_emitted += 1
                        other_pe += C_UNIT_PE
                    # spread batch-1 projections proportionally
                    while a_left > 0 and a_done_pe < TOTAL_A * other_pe / (
                            TOTAL_B0 + TOTAL_C_PH2):
                        next(gena1)
                        a_left -= 1
                        a_done_pe += A_UNIT_PE
                while a_left > 0:
                    next(gena1)
                    a_left -= 1
                # phase 3: batch-1 attention + remaining stage_c chunks
                c_queue.clear()  # b0 leftovers are tracked by c_tt below
                pend_c1 = []
                qco = (0, 1, 2, 3)
                genb1 = stage_b_units(t1[0], t1[1], t1[2], pend_c1,
                                      lbc_ab=True, qc_order=qco)
                c_tt = c_emitted  # next b0 tile index
                c1_pend = []
                c1_done = []
                kt_count = 0
                rounds_done = 0
                for item in genb1:
                    if item[0] == "round":
                        rounds_done += 1
                        qc_r = item[1]
                        for j in range(4):
                            c1_pend.append(qc_r * 4 + j)
                    kt_count += 1
                    if rounds_done >= 4:
                        continue
                    if kt_count % 2 == 0:
                        if c_tt < 16:
                            stage_c_unit(0, t0[2], c_tt, "ab")
                            c_tt += 1
                        elif c1_pend:
                            tt_ = c1_pend.pop(0)
                            stage_c_unit(1, t1[2], tt_, "ab")
                            c1_done.append(tt_)
                while c_tt < 16:
                    stage_c_unit(0, t0[2], c_tt, "ab")
                    c_tt += 1
                for tt_ in c1_pend:
                    stage_c_unit(1, t1[2], tt_, "ab")

            if reps == 1:
                body()
            else:
                with tc.For_i(0, reps, 1):
                    body()

    nc.compile()
    return nc


def make_inputs(x, Wq, Wk, Wv, Wo):
    """Host-side sharding/prep. Returns per-core input dicts."""
    x2 = np.ascontiguousarray(x.reshape(T, D))
    xt = np.ascontiguousarray(x2.T).astype(ml_dtypes.bfloat16)
    xt = np.ascontiguousarray(
        xt.reshape(16, 128, T // 256, 256).transpose(2, 1, 0, 3))

    inv_freq = 1.0 / (10000.0 ** (np.arange(0, DH, 2, dtype=np.float64) / DH))
    freqs = np.arange(S, dtype=np.float64)[:, None] * inv_freq[None, :]
    emb = np.concatenate([freqs, freqs], axis=1)
    cosT = np.cos(emb)[:, ::2].astype(np.float32)  # [S, 64]
    sinT = np.sin(emb)[:, ::2].astype(np.float32)
    c4 = np.ascontiguousarray(np.tile(cosT, (1, 4))).astype(ml_dtypes.bfloat16)
    s4 = np.ascontiguousarray(np.tile(sinT, (1, 4))).astype(ml_dtypes.bfloat16)
    lmask = (np.arange(128)[None, :] >= np.arange(128)[:, None]).astype(
        ml_dtypes.bfloat16)
    ones = np.ones((128, 128), ml_dtypes.bfloat16)

    in_maps = []
    for c in range(N_CORES):
        pr = []
        for h in (2 * c, 2 * c + 1):
            base = h * DH
            pr += [base + 2 * j for j in range(64)]
            pr += [base + 2 * j + 1 for j in range(64)]
        vr = list(range(2 * c * DH, 2 * c * DH + 2 * DH))
        wall = np.concatenate([Wq[pr].T, Wk[pr].T, Wv[vr].T],
                              axis=1).astype(ml_dtypes.bfloat16)
        w2 = np.ascontiguousarray(Wo[:, vr].T).astype(ml_dtypes.bfloat16)
        in_maps.append({
            "XT": xt, "WALL": wall, "W2": w2, "C4": c4, "S4": s4,
            "LM": lmask, "ONES": ones,
        })
    return in_maps


_NC_CACHE = {}


def kernel(x, Wq, Wk, Wv, Wo):
    x = np.asarray(x, dtype=np.float32)
    Wq = np.asarray(Wq, dtype=np.float32)
    Wk = np.asarray(Wk, dtype=np.float32)
    Wv = np.asarray(Wv, dtype=np.float32)
    Wo = np.asarray(Wo, dtype=np.float32)

    if 1 not in _NC_CACHE:
        _NC_CACHE[1] = build_nc(1)
    nc = _NC_CACHE[1]
    in_maps = make_inputs(x, Wq, Wk, Wv, Wo)
    import time as _time
    res = None
    for attempt in range(3):
        try:
            res = run_bass_kernel_spmd(nc, in_maps, core_ids=list(range(N_CORES)))
            break
        except Exception:
            if attempt == 2:
                raise
            _time.sleep(15)
    y = np.zeros((T, D), np.float64)
    for c in range(N_CORES):
        y += res.results[c]["Y"].astype(np.float64)
    return y.astype(np.float32).reshape(B, S, D)
